# revision 12
# baseline (speedup 1.0000x reference)
"""Averaged Hausdorff loss on 8 TRN2 NeuronCores — fp8 DoubleRow + exp-LSE.

Math: for X [N,64], Y [M,64]:
  loss = mean_n min_m d(n,m) + mean_m min_n d(n,m),  d = ||x_n - y_m||.
With S = -0.5 d^2 = x.y - 0.5||x||^2 - 0.5||y||^2:
  min_m d^2 = -2 max_m S (rows), min_n d^2 = -2 max_n S (cols).

Matmul: K=70 rows [x(64); 1,1,1; nx0..2] x [y(64); ny0..2; 1,1,1] where the
norm terms are residual-encoded into 3 fp8 rows each (abs err ~1e-3), inputs
quantized to fp8e4m3, and the PE runs fp8 DoubleRow (2x: K split 2x35).
Redundant per-matmul LDWEIGHTS (stationary unchanged within a row-tile) are
replaced by NoOps post-legalization — the PE keeps its stationary array.

Reduction: each core owns 2048 rows = 16 tiles of 128; per tile the 16384
columns form 8 psum groups of 2048.  Column group 0 is the "exact lane":
DVE evacuates it (tensor_copy to bf16 srow), max-folds into colS and
tree-reduces the exact per-row max over those 2048 cols.  Groups 1..7 are
the "LSE lane": the Act engine computes E = exp(gamma*(S - c)) straight
from PSUM (bf16 out) with fused accum_out = per-row sum of exp; DVE folds
each tile's E into colaccE with one wide bf16 TT (exp is monotonic, so the
column max commutes).  This keeps Act (7 groups) and DVE (1 group + bf16
folds) both ~14us/tile in one steady pipeline — no phase alternation.
Host combines: rowmax = max(exact lane, c + ln(rowsum)/gamma); column max
from colS (cols 0..2047) and c + ln(colE)/gamma (rest); sqrt + means.
gamma=2 with a sampled global bias c gives rel err ~2e-3 (validated in
numpy): no bf16 underflow for any column, softmin bias ~ -0.01 on term1.
"""

import numpy as np
import ml_dtypes

import concourse.bass as bass
import concourse.mybir as mybir
import concourse.tile as tile
from concourse.bass_utils import run_bass_kernel_spmd

N = 16384
M = 16384
D = 64
CORES = 8
ROWS_PER_CORE = N // CORES           # 2048
ROW_TILES = ROWS_PER_CORE // 128     # 16
GROUP = 2048                         # psum group width (4 banks f32)
GROUPS = M // GROUP                  # 8
MM_N = 512                           # moving cols per matmul (1 psum bank)
KH = 35                              # K = 70 = 2x35 for DoubleRow
CW = 3072                            # exact-lane cols (group 0 + half of 1)
EW = M - CW                          # LSE-lane width (13312)

GAMMA = 2.0

BF16 = mybir.dt.bfloat16
F32 = mybir.dt.float32
FP8 = mybir.dt.float8e4

mx = mybir.AluOpType.max

_CACHE: dict = {}

# walrus rejects instructions with more than one sync-wait command; excess
# waits are hoisted onto same-engine NOPs.
_MAX_WAITS = 1


def _split_excess_waits(nc: bass.Bass, cap: int = _MAX_WAITS) -> None:
    uid = [0]
    for fn in nc.m.functions:
        for bb in fn.blocks:
            out = []
            for inst in bb.instructions:
                si = inst.sync_info
                waits = list(si.on_wait) if si and si.on_wait else []
                if len(waits) > cap:
                    keep = waits[:cap]
                    extra = waits[cap:]
                    for w0 in range(0, len(extra), cap):
                        uid[0] += 1
                        nop = mybir.InstNoOp(
                            name=f"I-waitsplit-{uid[0]}",
                            engine=inst.engine,
                            bass_nofuse=True,
                            sync_info=mybir.SyncInfo(
                                on_wait=extra[w0:w0 + cap], on_update=[]),
                        )
                        nc.register_instruction(nop)
                        out.append(nop)
                    inst.sync_info = mybir.SyncInfo(
                        on_wait=keep, on_update=list(si.on_update))
                out.append(inst)
            bb.instructions[:] = out


def _dedup_ldweights(nc: bass.Bass) -> None:
    """Replace InstLdweights whose stationary matches the currently loaded
    weights with a same-engine NoOp carrying its sync_info (the PE array
    keeps the stationary across matmuls)."""
    uid = [0]
    for fn in nc.m.functions:
        for bb in fn.blocks:
            loaded = None
            out = []
            for inst in bb.instructions:
                if isinstance(inst, mybir.InstLdweights):
                    sig = repr(inst.ins[0])
                    if loaded == sig:
                        uid[0] += 1
                        nop = mybir.InstNoOp(
                            name=f"I-ldwdedup-{uid[0]}",
                            engine=inst.engine,
                            bass_nofuse=True,
                            sync_info=inst.sync_info,
                        )
                        nc.register_instruction(nop)
                        out.append(nop)
                        continue
                    loaded = sig
                out.append(inst)
            bb.instructions[:] = out


def _build_nc() -> bass.Bass:
    nc = bass.Bass()
    a_in = nc.declare_dram_parameter(
        "a", [KH, 2 * ROWS_PER_CORE], FP8, isOutput=False)
    b_in = nc.declare_dram_parameter("b", [KH, 2 * M], FP8, isOutput=False)
    cbias_in = nc.declare_dram_parameter("cbias", [128, 1], F32, isOutput=False)

    rowex_out = nc.declare_dram_parameter(
        "rowexact", [128, ROW_TILES], F32, isOutput=True)
    rowsum_out = nc.declare_dram_parameter(
        "rowsum", [128, ROW_TILES * (GROUPS - 1)], F32, isOutput=True)
    colE_out = nc.declare_dram_parameter("colE", [128, EW], BF16, isOutput=True)
    colS_out = nc.declare_dram_parameter("colS", [128, CW], BF16, isOutput=True)

    with tile.TileContext(nc) as tc:
        with (
            tc.tile_pool(name="const", bufs=1) as const,
            tc.tile_pool(name="acc", bufs=1) as acc,
            tc.tile_pool(name="ebuf", bufs=2) as ebuf_pool,
            tc.tile_pool(name="psum", bufs=2, space="PSUM") as psum_pool,
        ):
            a_sb = const.tile([KH, 2, ROWS_PER_CORE], FP8)
            nc.gpsimd.dma_start(
                a_sb[:], a_in[:].rearrange("p (t m) -> p t m", t=2))
            b_sb = const.tile([KH, 2, M], FP8)
            for jj in range(GROUPS):
                nc.sync.dma_start(
                    b_sb[:, :, jj * GROUP:(jj + 1) * GROUP],
                    b_in[:].rearrange("p (t m) -> p t m", t=2)
                    [:, :, jj * GROUP:(jj + 1) * GROUP])
            cbias = const.tile([128, 1], F32)
            nc.sync.dma_start(cbias[:], cbias_in[:])

            colE = acc.tile([128, EW], BF16)
            colS = acc.tile([128, CW], BF16)
            rowex = acc.tile([128, ROW_TILES], F32)
            rowsum = acc.tile([128, ROW_TILES * (GROUPS - 1)], F32)
            srow = acc.tile([128, CW], BF16)

            # etile/colE layout: E-lane col m (m >= CW) lives at index m - CW:
            # [0, 1024) from group 1's upper half, then 2048 per group 2..7.
            def esl_of(jj):
                if jj == 1:
                    return slice(0, 1024)
                return slice(1024 + (jj - 2) * GROUP, 1024 + (jj - 1) * GROUP)

            last = ROW_TILES - 1
            prev_etile = None
            # DVE-lane groups (0, 1) mid-tile so Act groups lead each tile
            # and the boundary psum fill overlaps the previous tile's exps.
            jj_order = [2, 3, 0, 4, 5, 1, 6, 7]
            for t in range(ROW_TILES):
                lhsT = a_sb[:, :, t * 128:(t + 1) * 128]
                etile = ebuf_pool.tile([128, EW], BF16, tag="etile")
                for jj in jj_order:
                    ps = psum_pool.tile([128, GROUP], F32, tag="ps")
                    for k in range(4):
                        c0 = jj * GROUP + k * MM_N
                        nc.tensor.matmul(
                            ps[:, k * MM_N:(k + 1) * MM_N],
                            lhsT,
                            b_sb[:, :, c0:c0 + MM_N],
                            start=True, stop=True,
                            perf_mode=mybir.MatmulPerfMode.DoubleRow)
                    if jj == 0:
                        # exact lane part 1: evacuate group 0 on DVE
                        nc.vector.tensor_copy(srow[:, :GROUP], ps[:])
                        continue
                    if jj == 1:
                        # exact lane part 2: lower half of group 1 on DVE
                        nc.vector.tensor_copy(srow[:, GROUP:CW], ps[:, :1024])
                        if t == 0:
                            nc.vector.tensor_copy(colS[:], srow[:])
                        else:
                            nc.vector.tensor_tensor(
                                out=colS[:], in0=colS[:], in1=srow[:], op=mx)
                        if t == last:
                            nc.sync.dma_start(colS_out[:], colS[:])
                        # LSE lane: upper half of group 1 on Act
                        nc.scalar.activation(
                            out=etile[:, esl_of(1)], in_=ps[:, 1024:],
                            func=mybir.ActivationFunctionType.Exp,
                            bias=cbias[:], scale=GAMMA,
                            accum_out=rowsum[:, t * (GROUPS - 1):
                                             t * (GROUPS - 1) + 1])
                        # exact rows: in-place tree on srow [128, 3072]
                        nc.vector.tensor_tensor(
                            out=srow[:, :1024], in0=srow[:, :1024],
                            in1=srow[:, 1024:2048], op=mx)
                        w = 1024
                        while w > 128:
                            nc.vector.tensor_tensor(
                                out=srow[:, :w], in0=srow[:, :w],
                                in1=srow[:, w:2 * w] if w != 1024
                                else srow[:, 2048:3072], op=mx)
                            w //= 2
                        nc.vector.tensor_tensor(
                            out=srow[:, :128], in0=srow[:, :128],
                            in1=srow[:, 128:256], op=mx)
                        nc.vector.tensor_reduce(
                            out=rowex[:, t:t + 1], in_=srow[:, :128],
                            axis=mybir.AxisListType.X, op=mx)
                    else:
                        nc.scalar.activation(
                            out=etile[:, esl_of(jj)], in_=ps[:],
                            func=mybir.ActivationFunctionType.Exp,
                            bias=cbias[:], scale=GAMMA,
                            accum_out=rowsum[:, t * (GROUPS - 1) + jj - 1:
                                             t * (GROUPS - 1) + jj])
                    if t == last:
                        # fold prev + this last-tile E chunk, ship immediately
                        esl = esl_of(jj)
                        nc.vector.tensor_tensor(
                            out=colE[:, esl], in0=colE[:, esl],
                            in1=prev_etile[:, esl], op=mx)
                        nc.vector.tensor_tensor(
                            out=colE[:, esl], in0=colE[:, esl],
                            in1=etile[:, esl], op=mx)
                        nc.sync.dma_start(colE_out[:, esl], colE[:, esl])
                    elif prev_etile is not None and jj in (5, 7):
                        # fold the PREVIOUS tile's E in two large chunks,
                        # placed after mid/late exps so the psum-critical
                        # copies stay at the head of DVE's queue.
                        esl = slice(0, 6144) if jj == 5 else slice(6144, EW)
                        if t == 1:
                            nc.vector.tensor_copy(
                                colE[:, esl], prev_etile[:, esl])
                        else:
                            nc.vector.tensor_tensor(
                                out=colE[:, esl], in0=colE[:, esl],
                                in1=prev_etile[:, esl], op=mx)
                prev_etile = etile

            nc.sync.dma_start(rowex_out[:], rowex[:])
            nc.sync.dma_start(rowsum_out[:], rowsum[:])

    _dedup_ldweights(nc)
    _split_excess_waits(nc)
    return nc


def get_nc() -> bass.Bass:
    if "nc" not in _CACHE:
        _CACHE["nc"] = _build_nc()
    return _CACHE["nc"]


def _enc_res(v: np.ndarray, k: int) -> np.ndarray:
    """Residual-encode v into k fp8 rows (summing to ~v)."""
    rows, acc = [], np.zeros_like(v)
    for _ in range(k):
        r = (v - acc).astype(ml_dtypes.float8_e4m3).astype(np.float32)
        rows.append(r)
        acc = acc + r
    return np.stack(rows)


def _prep(set1: np.ndarray, set2: np.ndarray):
    xq = np.asarray(set1, np.float32).astype(
        ml_dtypes.float8_e4m3).astype(np.float32)
    yq = np.asarray(set2, np.float32).astype(
        ml_dtypes.float8_e4m3).astype(np.float32)
    nx = -0.5 * np.einsum('nd,nd->n', xq, xq)
    ny = -0.5 * np.einsum('md,md->m', yq, yq)
    nxr = _enc_res(nx, 3)
    nyr = _enc_res(ny, 3)

    a70 = np.zeros((70, N), np.float32)
    a70[:D] = xq.T
    a70[D:D + 3] = 1.0
    a70[D + 3:D + 6] = nxr
    b70 = np.zeros((70, M), np.float32)
    b70[:D] = yq.T
    b70[D:D + 3] = nyr
    b70[D + 3:D + 6] = 1.0

    # global max-S estimate from a 512-row sample (+margin), for exp bias
    s_samp = a70[:, ::32].T @ b70  # [512, M]
    c = float(s_samp.max()) + 0.5
    return a70, b70, c


def make_in_maps(set1, set2):
    a70, b70, c = _prep(set1, set2)
    a8 = a70.reshape(2, KH, N).transpose(1, 0, 2)    # [35, 2, N]
    b8 = np.ascontiguousarray(
        b70.reshape(2, KH, M).transpose(1, 0, 2).reshape(KH, 2 * M)
    ).astype(ml_dtypes.float8_e4m3)
    cbias = np.full((128, 1), -GAMMA * c, np.float32)
    in_maps = []
    for core in range(CORES):
        asl = np.ascontiguousarray(
            a8[:, :, core * ROWS_PER_CORE:(core + 1) * ROWS_PER_CORE]
            .reshape(KH, 2 * ROWS_PER_CORE)).astype(ml_dtypes.float8_e4m3)
        in_maps.append({"a": asl, "b": b8, "cbias": cbias})
    return in_maps, c


def combine(results: list, c: float) -> np.float32:
    # term1: rows.  row n = core*2048 + t*128 + p.
    d2 = np.zeros(N, np.float32)
    for core, res in enumerate(results):
        rowex = np.asarray(res["rowexact"], np.float32)      # [128, 16]
        rowsum = np.asarray(res["rowsum"], np.float32)       # [128, 16*7]
        rs = rowsum.reshape(128, ROW_TILES, GROUPS - 1).sum(axis=2)
        for t in range(ROW_TILES):
            r0 = core * ROWS_PER_CORE + t * 128
            soft = c + np.log(np.maximum(rs[:, t], 1e-38)) / GAMMA
            smax = np.maximum(rowex[:, t], soft)
            d2[r0:r0 + 128] = np.maximum(-2.0 * smax, 0)
    term1 = np.sqrt(d2).mean()

    # term2: columns.  cols 0..CW-1 exact (colS), the rest via colE (exp).
    cE = np.zeros(EW, np.float32)
    cS = np.full(CW, -np.inf, np.float32)
    for res in results:
        cE = np.maximum(cE, np.asarray(res["colE"], np.float32).max(axis=0))
        cS = np.maximum(cS, np.asarray(res["colS"], np.float32).max(axis=0))
    colmax = np.empty(M, np.float32)
    colmax[:CW] = cS
    colmax[CW:] = np.where(
        cE > 0, c + np.log(np.maximum(cE, 1e-38)) / GAMMA, -np.inf)
    term2 = np.sqrt(np.maximum(-2.0 * colmax, 0)).mean()
    return np.float32(term1 + term2)


def run(set1, set2, trace: bool = False):
    nc = get_nc()
    in_maps, c = make_in_maps(set1, set2)
    res = run_bass_kernel_spmd(nc, in_maps, list(range(CORES)), trace=trace)
    return combine(res.results, c), res


def kernel(set1, set2) -> np.ndarray:
    out, _ = run(set1, set2, trace=False)
    return out


# revision 13
# speedup vs baseline: 1.1066x; 1.1066x over previous
"""Averaged Hausdorff loss on 8 TRN2 NeuronCores — fp8 DoubleRow + exp-LSE.

Math: for X [N,64], Y [M,64]:
  loss = mean_n min_m d(n,m) + mean_m min_n d(n,m),  d = ||x_n - y_m||.
With S = -0.5 d^2 = x.y - 0.5||x||^2 - 0.5||y||^2:
  min_m d^2 = -2 max_m S (rows), min_n d^2 = -2 max_n S (cols).

Matmul: K=70 rows [x(64); 1,1,1; nx0..2] x [y(64); ny0..2; 1,1,1] where the
norm terms are residual-encoded into 3 fp8 rows each (abs err ~1e-3), inputs
quantized to fp8e4m3, and the PE runs fp8 DoubleRow (2x: K split 2x35).
Redundant per-matmul LDWEIGHTS (stationary unchanged within a row-tile) are
replaced by NoOps post-legalization — the PE keeps its stationary array.

Reduction: each core owns 2048 rows = 16 tiles of 128; per tile the 16384
columns form 8 psum groups of 2048.  Column group 0 is the "exact lane":
DVE evacuates it (tensor_copy to bf16 srow), max-folds into colS and
tree-reduces the exact per-row max over those 2048 cols.  Groups 1..7 are
the "LSE lane": the Act engine computes E = exp(gamma*(S - c)) straight
from PSUM (bf16 out) with fused accum_out = per-row sum of exp; DVE folds
each tile's E into colaccE with one wide bf16 TT (exp is monotonic, so the
column max commutes).  This keeps Act (7 groups) and DVE (1 group + bf16
folds) both ~14us/tile in one steady pipeline — no phase alternation.
Host combines: rowmax = max(exact lane, c + ln(rowsum)/gamma); column max
from colS (cols 0..2047) and c + ln(colE)/gamma (rest); sqrt + means.
gamma=2 with a sampled global bias c gives rel err ~2e-3 (validated in
numpy): no bf16 underflow for any column, softmin bias ~ -0.01 on term1.
"""

import numpy as np
import ml_dtypes

import concourse.bass as bass
import concourse.mybir as mybir
import concourse.tile as tile
from concourse.bass_utils import run_bass_kernel_spmd

N = 16384
M = 16384
D = 64
CORES = 8
ROWS_PER_CORE = N // CORES           # 2048
ROW_TILES = ROWS_PER_CORE // 128     # 16
GROUP = 2048                         # psum group width (4 banks f32)
GROUPS = M // GROUP                  # 8
MM_N = 512                           # moving cols per matmul (1 psum bank)
KH = 35                              # K = 70 = 2x35 for DoubleRow
CW = GROUP                           # exact-lane cols (group 0)
EW = M - CW                          # LSE-lane width (14336)

GAMMA = 2.0

BF16 = mybir.dt.bfloat16
F32 = mybir.dt.float32
FP8 = mybir.dt.float8e4

mx = mybir.AluOpType.max

_CACHE: dict = {}

# walrus rejects instructions with more than one sync-wait command; excess
# waits are hoisted onto same-engine NOPs.
_MAX_WAITS = 1


def _split_excess_waits(nc: bass.Bass, cap: int = _MAX_WAITS) -> None:
    uid = [0]
    for fn in nc.m.functions:
        for bb in fn.blocks:
            out = []
            for inst in bb.instructions:
                si = inst.sync_info
                waits = list(si.on_wait) if si and si.on_wait else []
                if len(waits) > cap:
                    keep = waits[:cap]
                    extra = waits[cap:]
                    for w0 in range(0, len(extra), cap):
                        uid[0] += 1
                        nop = mybir.InstNoOp(
                            name=f"I-waitsplit-{uid[0]}",
                            engine=inst.engine,
                            bass_nofuse=True,
                            sync_info=mybir.SyncInfo(
                                on_wait=extra[w0:w0 + cap], on_update=[]),
                        )
                        nc.register_instruction(nop)
                        out.append(nop)
                    inst.sync_info = mybir.SyncInfo(
                        on_wait=keep, on_update=list(si.on_update))
                out.append(inst)
            bb.instructions[:] = out


def _dedup_ldweights(nc: bass.Bass) -> None:
    """Replace InstLdweights whose stationary matches the currently loaded
    weights with a same-engine NoOp carrying its sync_info (the PE array
    keeps the stationary across matmuls)."""
    uid = [0]
    for fn in nc.m.functions:
        for bb in fn.blocks:
            loaded = None
            out = []
            for inst in bb.instructions:
                if isinstance(inst, mybir.InstLdweights):
                    sig = repr(inst.ins[0])
                    if loaded == sig:
                        uid[0] += 1
                        nop = mybir.InstNoOp(
                            name=f"I-ldwdedup-{uid[0]}",
                            engine=inst.engine,
                            bass_nofuse=True,
                            sync_info=inst.sync_info,
                        )
                        nc.register_instruction(nop)
                        out.append(nop)
                        continue
                    loaded = sig
                out.append(inst)
            bb.instructions[:] = out


def _build_nc() -> bass.Bass:
    nc = bass.Bass()
    a_in = nc.declare_dram_parameter(
        "a", [KH, 2 * ROWS_PER_CORE], FP8, isOutput=False)
    b_in = nc.declare_dram_parameter("b", [KH, 2 * M], FP8, isOutput=False)
    cbias_in = nc.declare_dram_parameter("cbias", [128, 1], F32, isOutput=False)

    rowex_out = nc.declare_dram_parameter(
        "rowexact", [128, ROW_TILES], F32, isOutput=True)
    rowsum_out = nc.declare_dram_parameter(
        "rowsum", [128, ROW_TILES * (GROUPS - 1)], F32, isOutput=True)
    colE_out = nc.declare_dram_parameter("colE", [128, EW], BF16, isOutput=True)
    colS_out = nc.declare_dram_parameter("colS", [128, CW], BF16, isOutput=True)

    with tile.TileContext(nc) as tc:
        with (
            tc.tile_pool(name="const", bufs=1) as const,
            tc.tile_pool(name="acc", bufs=1) as acc,
            tc.tile_pool(name="ebuf", bufs=2) as ebuf_pool,
            tc.tile_pool(name="psum", bufs=2, space="PSUM") as psum_pool,
        ):
            a_sb = const.tile([KH, 2, ROWS_PER_CORE], FP8)
            nc.gpsimd.dma_start(
                a_sb[:], a_in[:].rearrange("p (t m) -> p t m", t=2))
            b_sb = const.tile([KH, 2, M], FP8)
            for jj in range(GROUPS):
                nc.sync.dma_start(
                    b_sb[:, :, jj * GROUP:(jj + 1) * GROUP],
                    b_in[:].rearrange("p (t m) -> p t m", t=2)
                    [:, :, jj * GROUP:(jj + 1) * GROUP])
            cbias = const.tile([128, 1], F32)
            nc.sync.dma_start(cbias[:], cbias_in[:])

            colE = acc.tile([128, EW], BF16)
            colS = acc.tile([128, CW], BF16)
            rowex = acc.tile([128, ROW_TILES], F32)
            rowsum = acc.tile([128, ROW_TILES * (GROUPS - 1)], F32)
            srow = acc.tile([128, CW], BF16)

            # etile/colE layout: E-lane col m (m >= CW) at index m - CW.
            def esl_of(jj):
                return slice((jj - 1) * GROUP, jj * GROUP)

            last = ROW_TILES - 1
            prev_etile = None
            # DVE-lane groups (0, 1) mid-tile so Act groups lead each tile
            # and the boundary psum fill overlaps the previous tile's exps.
            jj_order = [1, 2, 0, 3, 4, 5, 6, 7]
            for t in range(ROW_TILES):
                lhsT = a_sb[:, :, t * 128:(t + 1) * 128]
                etile = ebuf_pool.tile([128, EW], BF16, tag="etile")
                for jj in jj_order:
                    ps = psum_pool.tile([128, GROUP], F32, tag="ps")
                    for k in range(4):
                        c0 = jj * GROUP + k * MM_N
                        nc.tensor.matmul(
                            ps[:, k * MM_N:(k + 1) * MM_N],
                            lhsT,
                            b_sb[:, :, c0:c0 + MM_N],
                            start=True, stop=True,
                            perf_mode=mybir.MatmulPerfMode.DoubleRow)
                    if jj == 0:
                        # exact lane: evacuate group 0, fold colS, tree rows
                        nc.vector.tensor_copy(srow[:], ps[:])
                        if t == 0:
                            nc.vector.tensor_copy(colS[:], srow[:])
                        else:
                            nc.vector.tensor_tensor(
                                out=colS[:], in0=colS[:], in1=srow[:], op=mx)
                        if t == last:
                            nc.sync.dma_start(colS_out[:], colS[:])
                        # exact rows: in-place tree on srow [128, 2048]
                        w = CW // 2
                        while w > 128:
                            nc.vector.tensor_tensor(
                                out=srow[:, :w], in0=srow[:, :w],
                                in1=srow[:, w:2 * w], op=mx)
                            w //= 2
                        nc.vector.tensor_tensor(
                            out=srow[:, :128], in0=srow[:, :128],
                            in1=srow[:, 128:256], op=mx)
                        nc.vector.tensor_reduce(
                            out=rowex[:, t:t + 1], in_=srow[:, :128],
                            axis=mybir.AxisListType.X, op=mx)
                        continue
                    nc.scalar.activation(
                        out=etile[:, esl_of(jj)], in_=ps[:],
                        func=mybir.ActivationFunctionType.Exp,
                        bias=cbias[:], scale=GAMMA,
                        accum_out=rowsum[:, t * (GROUPS - 1) + jj - 1:
                                         t * (GROUPS - 1) + jj])
                    if t == last:
                        # fold prev + this last-tile E chunk, ship immediately
                        esl = esl_of(jj)
                        nc.vector.tensor_tensor(
                            out=colE[:, esl], in0=colE[:, esl],
                            in1=prev_etile[:, esl], op=mx)
                        nc.vector.tensor_tensor(
                            out=colE[:, esl], in0=colE[:, esl],
                            in1=etile[:, esl], op=mx)
                        nc.sync.dma_start(colE_out[:, esl], colE[:, esl])
                    elif prev_etile is not None and jj in (5, 7):
                        # fold the PREVIOUS tile's E in two large chunks,
                        # placed after mid/late exps so the psum-critical
                        # copy stays at the head of DVE's queue.
                        esl = slice(0, 7168) if jj == 5 else slice(7168, EW)
                        if t == 1:
                            nc.vector.tensor_copy(
                                colE[:, esl], prev_etile[:, esl])
                        else:
                            nc.vector.tensor_tensor(
                                out=colE[:, esl], in0=colE[:, esl],
                                in1=prev_etile[:, esl], op=mx)
                prev_etile = etile

            nc.sync.dma_start(rowex_out[:], rowex[:])
            nc.sync.dma_start(rowsum_out[:], rowsum[:])

    _dedup_ldweights(nc)
    _split_excess_waits(nc)
    return nc


def get_nc() -> bass.Bass:
    if "nc" not in _CACHE:
        _CACHE["nc"] = _build_nc()
    return _CACHE["nc"]


def _enc_res(v: np.ndarray, k: int) -> np.ndarray:
    """Residual-encode v into k fp8 rows (summing to ~v)."""
    rows, acc = [], np.zeros_like(v)
    for _ in range(k):
        r = (v - acc).astype(ml_dtypes.float8_e4m3).astype(np.float32)
        rows.append(r)
        acc = acc + r
    return np.stack(rows)


def _prep(set1: np.ndarray, set2: np.ndarray):
    xq = np.asarray(set1, np.float32).astype(
        ml_dtypes.float8_e4m3).astype(np.float32)
    yq = np.asarray(set2, np.float32).astype(
        ml_dtypes.float8_e4m3).astype(np.float32)
    nx = -0.5 * np.einsum('nd,nd->n', xq, xq)
    ny = -0.5 * np.einsum('md,md->m', yq, yq)
    nxr = _enc_res(nx, 3)
    nyr = _enc_res(ny, 3)

    a70 = np.zeros((70, N), np.float32)
    a70[:D] = xq.T
    a70[D:D + 3] = 1.0
    a70[D + 3:D + 6] = nxr
    b70 = np.zeros((70, M), np.float32)
    b70[:D] = yq.T
    b70[D:D + 3] = nyr
    b70[D + 3:D + 6] = 1.0

    # global max-S estimate from a 512-row sample (+margin), for exp bias
    s_samp = a70[:, ::32].T @ b70  # [512, M]
    c = float(s_samp.max()) + 0.5
    return a70, b70, c


def make_in_maps(set1, set2):
    a70, b70, c = _prep(set1, set2)
    a8 = a70.reshape(2, KH, N).transpose(1, 0, 2)    # [35, 2, N]
    b8 = np.ascontiguousarray(
        b70.reshape(2, KH, M).transpose(1, 0, 2).reshape(KH, 2 * M)
    ).astype(ml_dtypes.float8_e4m3)
    cbias = np.full((128, 1), -GAMMA * c, np.float32)
    in_maps = []
    for core in range(CORES):
        asl = np.ascontiguousarray(
            a8[:, :, core * ROWS_PER_CORE:(core + 1) * ROWS_PER_CORE]
            .reshape(KH, 2 * ROWS_PER_CORE)).astype(ml_dtypes.float8_e4m3)
        in_maps.append({"a": asl, "b": b8, "cbias": cbias})
    return in_maps, c


def combine(results: list, c: float) -> np.float32:
    # term1: rows.  row n = core*2048 + t*128 + p.
    d2 = np.zeros(N, np.float32)
    for core, res in enumerate(results):
        rowex = np.asarray(res["rowexact"], np.float32)      # [128, 16]
        rowsum = np.asarray(res["rowsum"], np.float32)       # [128, 16*7]
        rs = rowsum.reshape(128, ROW_TILES, GROUPS - 1).sum(axis=2)
        for t in range(ROW_TILES):
            r0 = core * ROWS_PER_CORE + t * 128
            soft = c + np.log(np.maximum(rs[:, t], 1e-38)) / GAMMA
            smax = np.maximum(rowex[:, t], soft)
            d2[r0:r0 + 128] = np.maximum(-2.0 * smax, 0)
    term1 = np.sqrt(d2).mean()

    # term2: columns.  cols 0..CW-1 exact (colS), the rest via colE (exp).
    cE = np.zeros(EW, np.float32)
    cS = np.full(CW, -np.inf, np.float32)
    for res in results:
        cE = np.maximum(cE, np.asarray(res["colE"], np.float32).max(axis=0))
        cS = np.maximum(cS, np.asarray(res["colS"], np.float32).max(axis=0))
    colmax = np.empty(M, np.float32)
    colmax[:CW] = cS
    colmax[CW:] = np.where(
        cE > 0, c + np.log(np.maximum(cE, 1e-38)) / GAMMA, -np.inf)
    term2 = np.sqrt(np.maximum(-2.0 * colmax, 0)).mean()
    return np.float32(term1 + term2)


def run(set1, set2, trace: bool = False):
    nc = get_nc()
    in_maps, c = make_in_maps(set1, set2)
    res = run_bass_kernel_spmd(nc, in_maps, list(range(CORES)), trace=trace)
    return combine(res.results, c), res


def kernel(set1, set2) -> np.ndarray:
    out, _ = run(set1, set2, trace=False)
    return out
